# revision 25
# baseline (speedup 1.0000x reference)
"""Distributed causal multi-head attention for one TRN2 chip (8 NeuronCores).

Sharding: 2 batches x 4 head-groups. Core c handles batch c//4 and heads
[4*(c%4), 4*(c%4)+4). Wq/Wk/Wv are column-sliced per head group, Wo is
row-sliced; each core emits a partial (S, D) output and the host sums the
4 partials per batch and adds bo. No on-device collectives.

Per-core pipeline (PE cost = moving-dim rows; K/M idle halves recovered
via 32-aligned row/col tiling so paired matmuls run concurrently):
  Q^T,K^T = Wq/Wk-pairs.T @ X^T-pairs   fp8e4 DoubleRow (2 k-chunks/mm)
  V       = X^T-chunk.T @ Wv            bf16 (+ rank-1 bias matmul)
  S^T     = K_h^T.T @ Q_h^T             f32r, head pair row-tiled (0/64)
  P       = exp(scale * S^T)            ACT, one call per chunk (both heads)
  O^T     = V_h.T @ P_h                 bf16, head pair col-tiled (0/64)
  denom   = 1.T @ P_h                   bf16, 4 heads col-tiled (0/32/64/96)
  out     = (O^T/denom).T @ Wo          bf16 stationary, f32 out
"""

import math
import os
import sys

import numpy as np

for _p in ("/opt/trn_rl_repo", "/root/.axon_site/_ro/trn_rl_repo"):
    if os.path.isdir(_p) and _p not in sys.path:
        sys.path.insert(0, _p)

import ml_dtypes

import concourse.bass as bass
import concourse.mybir as mybir
import concourse.tile as tile
from concourse import bacc
from concourse.bass_utils import run_bass_kernel_spmd

F32 = mybir.dt.float32
F32R = mybir.dt.float32r
BF16 = mybir.dt.bfloat16
FP8 = mybir.dt.float8e4
AF = mybir.ActivationFunctionType
ALU = mybir.AluOpType
DR = mybir.MatmulPerfMode.DoubleRow

B, S, D = 2, 2048, 1024
H, HD = 16, 64
NCORES, NGROUPS = 8, 4
GC = D // NGROUPS            # 256 attention columns per core (4 heads)
GH = GC // HD                # 4 heads per core (2 pairs)
SCALE = 1.0 / math.sqrt(S)   # reference divides by sqrt(Sk), not sqrt(hd)
DCH = D // 128               # 8 contraction chunks over model dim
NPAIR = DCH // 2             # 4 DoubleRow k-chunk pairs
ST = S // 128                # 16 key chunks
NJ = S // 512                # 4 query blocks


def _emit(tc, xq8, xk8, xv16, wq8, wk8, wv16, bq, bk, bv16, wo16, out, dbg=None,
          pre_x=None):
    nc = tc.nc
    from contextlib import ExitStack

    with ExitStack() as ctx:
        const = ctx.enter_context(tc.tile_pool(name="const", bufs=1))
        perm = ctx.enter_context(tc.tile_pool(name="perm", bufs=1))
        xch = ctx.enter_context(tc.tile_pool(name="xch", bufs=8))
        ptp = ctx.enter_context(tc.tile_pool(name="ptp", bufs=4))
        nrm = ctx.enter_context(tc.tile_pool(name="nrm", bufs=4))
        outp = ctx.enter_context(tc.tile_pool(name="outp", bufs=3))
        # PSUM: sp 2-bank tiles x2 + op 1x2 + dn 1x2 = 8 banks
        if dbg is not None:
            dbgp = ctx.enter_context(tc.tile_pool(name="dbgp", bufs=1))
        sp = ctx.enter_context(tc.tile_pool(name="sp", bufs=2, space="PSUM"))
        op = ctx.enter_context(tc.tile_pool(name="op", bufs=2, space="PSUM"))
        dn = ctx.enter_context(tc.tile_pool(name="dn", bufs=2, space="PSUM"))

        # ---- constants ---------------------------------------------------
        def load_const(dram, shape, dt, tag):
            t = const.tile(shape, dt, tag=tag, name=tag)
            nc.sync.dma_start(out=t[:], in_=dram.ap())
            return t

        ones_col = const.tile([128, 1], BF16, tag="ones_c", name="ones_c")
        nc.vector.memset(ones_col[:], 1.0)
        ones_row = const.tile([1, 128], BF16, tag="ones_r", name="ones_r")
        nc.vector.memset(ones_row[:], 1.0)

        # persistent activation tiles --------------------------------------
        qt = [[perm.tile([128, 512], F32R, tag=f"qt{m}_{j}", name=f"qt{m}_{j}")
               for j in range(NJ)] for m in range(2)]
        kt = [[perm.tile([128, 512], F32R, tag=f"kt{m}_{j}", name=f"kt{m}_{j}")
               for j in range(NJ)] for m in range(2)]
        ot = [[perm.tile([128, 512], BF16, tag=f"ot{c}_{j}", name=f"ot{c}_{j}")
               for j in range(NJ)] for c in range(2)]
        vt = [perm.tile([128, GH, HD], BF16, tag=f"vt{t}", name=f"vt{t}")
              for t in range(ST)]

        # ---- QK projections (fp8 DoubleRow) ------------------------------
        wk_sb = load_const(wk8, [128, NPAIR, 2, 2, 128], FP8, "wk")
        bk_sb = const.tile([128, 2], F32, tag="bk", name="bk")
        nc.sync.dma_start(out=bk_sb[:], in_=bk.ap().rearrange("(m p) -> p m", p=128))

        def load_x8(dram, tag):
            tiles = []
            for p in range(NPAIR):
                t = xch.tile([128, 2, S], FP8, tag="x8", name=tag)
                nc.sync.dma_start(out=t[:], in_=dram[:, p, :, :])
                tiles.append(t)
            return tiles

        def proj_qk(xt, w_sb, b_sb, dst):
            for m in range(2):
                for j in range(NJ):
                    ps = op.tile([128, 512], F32, tag="op", name="pqk")
                    for p in range(NPAIR):
                        nc.tensor.matmul(
                            ps[:],
                            w_sb[:, p, :, m, :],
                            xt[p][:, :, 512 * j:512 * (j + 1)],
                            start=(p == 0), stop=(p == NPAIR - 1),
                            perf_mode=DR)
                    nc.vector.tensor_scalar_add(
                        out=dst[m][j][:], in0=ps[:], scalar1=b_sb[:, m:m + 1])

        xk_t = pre_x[1] if pre_x else load_x8(xk8, "xk")
        wq_sb = load_const(wq8, [128, NPAIR, 2, 2, 128], FP8, "wq")
        bq_sb = const.tile([128, 2], F32, tag="bq", name="bq")
        nc.sync.dma_start(out=bq_sb[:], in_=bq.ap().rearrange("(m p) -> p m", p=128))
        proj_qk(xk_t, wk_sb, bk_sb, kt)
        xq_t = pre_x[0] if pre_x else load_x8(xq8, "xq")
        proj_qk(xq_t, wq_sb, bq_sb, qt)

        # ---- V projection (bf16) -----------------------------------------
        wv_sb = load_const(wv16, [128, DCH, GC], BF16, "wv")
        bv_sb = load_const(bv16, [1, GC], BF16, "bv")
        if pre_x:
            xv_t = pre_x[2]
        else:
            xv_t = []
            for d in range(DCH):
                t = xch.tile([128, S], BF16, tag="xv", name="xv")
                nc.sync.dma_start(out=t[:], in_=xv16[:, d, :])
                xv_t.append(t)
        for t in range(ST):
            ps = dn.tile([128, GC], F32, tag="dn", name="pv")
            for d in range(DCH):
                nc.tensor.matmul(
                    ps[:],
                    xv_t[d][:, 128 * t:128 * (t + 1)],
                    wv_sb[:, d, :],
                    start=(d == 0), stop=False)
            nc.tensor.matmul(ps[:], ones_row[:], bv_sb[:], start=False, stop=True)
            nc.vector.tensor_copy(
                out=vt[t][:], in_=ps[:].rearrange("p (h e) -> p h e", h=GH))

        wo_sb = load_const(wo16, [128, 2, D], BF16, "wo")

        # ---- attention ---------------------------------------------------
        for j in range(NJ):
            nch = 4 * (j + 1)
            dnt = dn.tile([128, 512], F32, tag="dn", name="dnt")
            otp = [op.tile([128, 512], F32, tag="op", name=f"otp{c}")
                   for c in range(2)]
            for i in range(nch):
                o = i - 4 * j
                off = 0 if o < 0 else min(128 * o, 256)
                pts = []
                for c in range(2):
                    spt = sp.tile([128, 1024], F32, tag="sp", name="spt")
                    for h2 in range(2):
                        nc.tensor.matmul(
                            spt[:, 512 * h2 + off:512 * (h2 + 1)],
                            kt[c][i // 4][64 * h2:64 * (h2 + 1),
                                          128 * (i % 4):128 * (i % 4) + 128],
                            qt[c][j][64 * h2:64 * (h2 + 1), off:512],
                            start=True, stop=True)
                    pt = ptp.tile([128, 1024], BF16, tag="pt", name="ptt")
                    if o == 3:
                        # exact spans: cols [384,512) and [896,1024)
                        for h2 in range(2):
                            nc.scalar.activation(
                                out=pt[:, 512 * h2 + 384:512 * (h2 + 1)],
                                in_=spt[:, 512 * h2 + 384:512 * (h2 + 1)],
                                func=AF.Exp, scale=SCALE)
                    else:
                        nc.scalar.activation(
                            out=pt[:, off:1024], in_=spt[:, off:1024],
                            func=AF.Exp, scale=SCALE)
                    if o >= 0:
                        # zero the sub-diagonal triangle in both head halves
                        w = 512 - off if o == 3 else 128
                        base = 0 if o < 3 else off - 128 * o
                        nc.gpsimd.affine_select(
                            out=pt[:].rearrange("p (h q) -> p h q", h=2)
                                   [:, :, off:off + w],
                            in_=pt[:].rearrange("p (h q) -> p h q", h=2)
                                   [:, :, off:off + w],
                            compare_op=ALU.is_ge, fill=0.0, base=base,
                            pattern=[[0, 2], [1, w]], channel_multiplier=-1)
                    pts.append(pt)
                    if dbg is not None and j == 0 and c == 0:
                        nc.sync.dma_start(out=dbg["pt"][:, i, :], in_=pt[:])
                    for h2 in range(2):
                        nc.tensor.matmul(
                            otp[c][64 * h2:64 * (h2 + 1), off:512],
                            vt[i][:, 2 * c + h2, :],
                            pt[:, 512 * h2 + off:512 * (h2 + 1)],
                            start=(i == 0), stop=(i == nch - 1),
                            skip_group_check=True)
                for c in range(2):
                    for h2 in range(2):
                        nc.tensor.matmul(
                            dnt[64 * c + 32 * h2:64 * c + 32 * h2 + 1, off:512],
                            ones_col[:],
                            pts[c][:, 512 * h2 + off:512 * (h2 + 1)],
                            start=(i == 0), stop=(i == nch - 1),
                            skip_group_check=True,
                            tile_position=(0, 64 * c + 32 * h2))
            # normalize: O^T / denom -> ot (bf16)
            if dbg is not None and j == 0:
                dcp = dbgp.tile([128, 512], F32, tag="dcp", name="dcp")
                nc.vector.tensor_copy(out=dcp[:], in_=dnt[:])
                nc.sync.dma_start(out=dbg["dn0"].ap(), in_=dcp[:])
                for c in range(2):
                    ocp = dbgp.tile([128, 512], F32, tag=f"ocp{c}", name="ocp")
                    nc.vector.tensor_copy(out=ocp[:], in_=otp[c][:])
                    nc.sync.dma_start(out=dbg["op0"][:, c, :], in_=ocp[:])
            rcs = []
            for hh in range(4):
                rc = nrm.tile([1, 512], F32, tag=f"rc{hh}", name=f"rc{hh}")
                nc.vector.reciprocal(rc[:], dnt[32 * hh:32 * hh + 1, :])
                rcs.append(rc)
            if dbg is not None and j == 0:
                for hh in range(4):
                    nc.sync.dma_start(out=dbg["rc0"][hh:hh + 1, :], in_=rcs[hh][:])
            for c in range(2):
                for h2 in range(2):
                    bc = nrm.tile([64, 512], F32, tag=f"bc{h2}", name="bc")
                    nc.gpsimd.partition_broadcast(bc[:], rcs[2 * c + h2][:])
                    nc.vector.tensor_tensor(
                        out=ot[c][j][64 * h2:64 * (h2 + 1), :],
                        in0=otp[c][64 * h2:64 * (h2 + 1), :],
                        in1=bc[:], op=ALU.mult)

        if dbg is not None:
            for m in range(2):
                for j in range(NJ):
                    nc.sync.dma_start(
                        out=dbg["qt"][:, m, 512 * j:512 * (j + 1)],
                        in_=qt[m][j][:].bitcast(F32))
                    nc.sync.dma_start(
                        out=dbg["kt"][:, m, 512 * j:512 * (j + 1)],
                        in_=kt[m][j][:].bitcast(F32))
                    nc.sync.dma_start(
                        out=dbg["ot"][:, m, 512 * j:512 * (j + 1)],
                        in_=ot[m][j][:])
            for t in range(ST):
                nc.sync.dma_start(out=dbg["vt"][:, t, :, :], in_=vt[t][:])

        # ---- output projection -------------------------------------------
        for j in range(NJ):
            for tt in range(4):
                t = 4 * j + tt
                ob = outp.tile([128, D], F32, tag="ob", name="ob")
                for n2 in range(2):
                    fpt = op.tile([128, 512], F32, tag="op", name="fpt")
                    for c in range(2):
                        nc.tensor.matmul(
                            fpt[:],
                            ot[c][j][:, 128 * tt:128 * (tt + 1)],
                            wo_sb[:, c, 512 * n2:512 * (n2 + 1)],
                            start=(c == 0), stop=(c == 1))
                    nc.vector.tensor_copy(out=ob[:, 512 * n2:512 * (n2 + 1)], in_=fpt[:])
                nc.sync.dma_start(out=out[128 * t:128 * (t + 1), :], in_=ob[:])


_PROGRAMS = {}


def _build_program(reps=1, loop=0, debug_dumps=False, hoist_x=False):
    nc = bacc.Bacc("TRN2", target_bir_lowering=False, debug=False,
                   num_devices=NCORES)
    xq8 = nc.declare_dram_parameter("xq8", [128, NPAIR, 2, S], FP8, isOutput=False)
    xk8 = nc.declare_dram_parameter("xk8", [128, NPAIR, 2, S], FP8, isOutput=False)
    xv16 = nc.declare_dram_parameter("xv16", [128, DCH, S], BF16, isOutput=False)
    wq8 = nc.declare_dram_parameter("wq8", [128, NPAIR, 2, 2, 128], FP8, isOutput=False)
    wk8 = nc.declare_dram_parameter("wk8", [128, NPAIR, 2, 2, 128], FP8, isOutput=False)
    wv16 = nc.declare_dram_parameter("wv16", [128, DCH, GC], BF16, isOutput=False)
    bq = nc.declare_dram_parameter("bq", [GC], F32, isOutput=False)
    bk = nc.declare_dram_parameter("bk", [GC], F32, isOutput=False)
    bv16 = nc.declare_dram_parameter("bv16", [1, GC], BF16, isOutput=False)
    wo16 = nc.declare_dram_parameter("wo16", [128, 2, D], BF16, isOutput=False)
    out = nc.declare_dram_parameter("out_p", [S, D], F32, isOutput=True)
    dbg = None
    if debug_dumps:
        dbg = {
            "qt": nc.declare_dram_parameter("dbg_qt", [128, 2, S], F32, isOutput=True),
            "kt": nc.declare_dram_parameter("dbg_kt", [128, 2, S], F32, isOutput=True),
            "ot": nc.declare_dram_parameter("dbg_ot", [128, 2, S], BF16, isOutput=True),
            "vt": nc.declare_dram_parameter("dbg_vt", [128, ST, GH, HD], BF16, isOutput=True),
            "pt": nc.declare_dram_parameter("dbg_pt", [128, 4, 1024], BF16, isOutput=True),
            "dn0": nc.declare_dram_parameter("dbg_dn0", [128, 512], F32, isOutput=True),
            "rc0": nc.declare_dram_parameter("dbg_rc0", [4, 512], F32, isOutput=True),
            "op0": nc.declare_dram_parameter("dbg_op0", [128, 2, 512], F32, isOutput=True),
        }
    with tile.TileContext(nc) as tc:
        pre_x = None
        if hoist_x:
            with tc.tile_pool(name="prex", bufs=8) as prex:
                xq_t, xk_t = [], []
                for dram, lst in ((xq8, xq_t), (xk8, xk_t)):
                    for p in range(NPAIR):
                        t = prex.tile([128, 2, S], FP8, tag="x8", name="px8")
                        nc.sync.dma_start(out=t[:], in_=dram[:, p, :, :])
                        lst.append(t)
                xv_t = []
                for d in range(DCH):
                    t = prex.tile([128, S], BF16, tag="xv", name="pxv")
                    nc.sync.dma_start(out=t[:], in_=xv16[:, d, :])
                    xv_t.append(t)
                pre_x = (xq_t, xk_t, xv_t)
                if loop:
                    with tc.For_i(0, loop, 1):
                        _emit(tc, xq8, xk8, xv16, wq8, wk8, wv16, bq, bk, bv16,
                              wo16, out, pre_x=pre_x)
                else:
                    for _ in range(reps):
                        _emit(tc, xq8, xk8, xv16, wq8, wk8, wv16, bq, bk, bv16,
                              wo16, out, dbg=dbg, pre_x=pre_x)
        elif loop:
            with tc.For_i(0, loop, 1):
                _emit(tc, xq8, xk8, xv16, wq8, wk8, wv16, bq, bk, bv16, wo16, out)
        else:
            for _ in range(reps):
                _emit(tc, xq8, xk8, xv16, wq8, wk8, wv16, bq, bk, bv16, wo16, out,
                      dbg=dbg)
    nc.compile()
    return nc


def get_program(reps=1, loop=0, debug_dumps=False, hoist_x=False):
    key = (reps, loop, debug_dumps, hoist_x)
    if key not in _PROGRAMS:
        _PROGRAMS[key] = _build_program(reps, loop, debug_dumps, hoist_x)
    return _PROGRAMS[key]


def _to_fp8_pairs(xT):
    # xT: (D, S) f32 -> [128, NPAIR, 2, S] fp8e4: out[k, p, s, t] = xT[256p+128s+k, t]
    a = xT.reshape(NPAIR, 2, 128, -1).transpose(2, 0, 1, 3)
    return np.ascontiguousarray(a.astype(ml_dtypes.float8_e4m3fn))


def _w_fp8_pairs(W):
    # W: (D, GC) f32 -> [128, NPAIR, 2, 2, 128]: out[k,p,s,m,c] = W[256p+128s+k, 128m+c]
    a = W.reshape(NPAIR, 2, 128, 2, 128).transpose(2, 0, 1, 3, 4)
    return np.ascontiguousarray(a.astype(ml_dtypes.float8_e4m3fn))


def make_in_maps(inputs):
    q = np.asarray(inputs["query"], np.float32)
    k = np.asarray(inputs["key"], np.float32)
    v = np.asarray(inputs["value"], np.float32)
    Wq = np.asarray(inputs["Wq"], np.float32)
    Wk = np.asarray(inputs["Wk"], np.float32)
    Wv = np.asarray(inputs["Wv"], np.float32)
    bq = np.asarray(inputs["bq"], np.float32)
    bk = np.asarray(inputs["bk"], np.float32)
    bv = np.asarray(inputs["bv"], np.float32)
    Wo = np.asarray(inputs["Wo"], np.float32)
    bf16 = ml_dtypes.bfloat16
    xq8 = [_to_fp8_pairs(q[b].T) for b in range(B)]
    xk8 = [_to_fp8_pairs(k[b].T) for b in range(B)]
    xv16 = [np.ascontiguousarray(
        v[b].T.reshape(DCH, 128, S).transpose(1, 0, 2).astype(bf16))
        for b in range(B)]
    in_maps = []
    for core in range(NCORES):
        b, g = divmod(core, NGROUPS)
        cs = slice(g * GC, (g + 1) * GC)
        in_maps.append({
            "xq8": xq8[b], "xk8": xk8[b], "xv16": xv16[b],
            "wq8": _w_fp8_pairs(Wq[:, cs]),
            "wk8": _w_fp8_pairs(Wk[:, cs]),
            "wv16": np.ascontiguousarray(
                Wv[:, cs].reshape(DCH, 128, GC).transpose(1, 0, 2).astype(bf16)),
            "bq": np.ascontiguousarray(bq[cs]),
            "bk": np.ascontiguousarray(bk[cs]),
            "bv16": np.ascontiguousarray(bv[cs].reshape(1, GC).astype(bf16)),
            "wo16": np.ascontiguousarray(
                Wo[cs, :].reshape(2, 128, D).transpose(1, 0, 2).astype(bf16)),
        })
    return in_maps


def combine_outputs(results, inputs):
    bo = np.asarray(inputs["bo"], np.float32)
    out = np.zeros((B, S, D), np.float32)
    for core in range(NCORES):
        out[core // NGROUPS] += results[core]["out_p"]
    out += bo
    return out


def kernel(**inputs):
    nc = get_program()
    in_maps = make_in_maps(inputs)
    res = run_bass_kernel_spmd(nc, in_maps, core_ids=list(range(NCORES)))
    return combine_outputs(res.results, inputs)


# revision 27
# speedup vs baseline: 1.3584x; 1.3584x over previous
"""Distributed causal multi-head attention for one TRN2 chip (8 NeuronCores).

Sharding: 2 batches x 4 head-groups. Core c handles batch c//4 and heads
[4*(c%4), 4*(c%4)+4). Wq/Wk/Wv are column-sliced per head group, Wo is
row-sliced; each core emits a partial (S, D) output and the host sums the
4 partials per batch and adds bo. No on-device collectives.

Per-core pipeline (PE cost = moving-dim rows; K/M idle halves recovered
via 32-aligned row/col tiling so paired matmuls run concurrently):
  Q^T,K^T = Wq/Wk-pairs.T @ X^T-pairs   fp8e4 DoubleRow (2 k-chunks/mm)
  V       = X^T-chunk.T @ Wv            bf16 (+ rank-1 bias matmul)
  S^T     = K_h^T.T @ Q_h^T             f32r, head pair row-tiled (0/64)
  P       = exp(scale * S^T)            ACT, one call per chunk (both heads)
  O^T     = V_h.T @ P_h                 bf16, head pair col-tiled (0/64)
  denom   = 1.T @ P_h                   bf16, 4 heads col-tiled (0/32/64/96)
  out     = (O^T/denom).T @ Wo          bf16 stationary, f32 out
"""

import math
import os
import sys

import numpy as np

for _p in ("/opt/trn_rl_repo", "/root/.axon_site/_ro/trn_rl_repo"):
    if os.path.isdir(_p) and _p not in sys.path:
        sys.path.insert(0, _p)

import ml_dtypes

import concourse.bass as bass
import concourse.mybir as mybir
import concourse.tile as tile
from concourse import bacc
from concourse.bass_utils import run_bass_kernel_spmd

F32 = mybir.dt.float32
F32R = mybir.dt.float32r
BF16 = mybir.dt.bfloat16
FP8 = mybir.dt.float8e4
AF = mybir.ActivationFunctionType
ALU = mybir.AluOpType
DR = mybir.MatmulPerfMode.DoubleRow

B, S, D = 2, 2048, 1024
H, HD = 16, 64
NCORES, NGROUPS = 8, 4
GC = D // NGROUPS            # 256 attention columns per core (4 heads)
GH = GC // HD                # 4 heads per core (2 pairs)
SCALE = 1.0 / math.sqrt(S)   # reference divides by sqrt(Sk), not sqrt(hd)
DCH = D // 128               # 8 contraction chunks over model dim
NPAIR = DCH // 2             # 4 DoubleRow k-chunk pairs
ST = S // 128                # 16 key chunks
NJ = S // 512                # 4 query blocks


def _emit(tc, xq8, xk8, xv16, wq8, wk8, wv16, bq, bk, bv16, wo16, out, dbg=None,
          pre_x=None):
    nc = tc.nc
    from contextlib import ExitStack

    with ExitStack() as ctx:
        const = ctx.enter_context(tc.tile_pool(name="const", bufs=1))
        perm = ctx.enter_context(tc.tile_pool(name="perm", bufs=1))
        xch = ctx.enter_context(tc.tile_pool(name="xch", bufs=8))
        ptp = ctx.enter_context(tc.tile_pool(name="ptp", bufs=6))
        nrm = ctx.enter_context(tc.tile_pool(name="nrm", bufs=4))
        outp = ctx.enter_context(tc.tile_pool(name="outp", bufs=3))
        # PSUM: sp 2-bank tiles x2 + op 1x2 + dn 1x2 = 8 banks
        if dbg is not None:
            dbgp = ctx.enter_context(tc.tile_pool(name="dbgp", bufs=1))
        sp = ctx.enter_context(tc.tile_pool(name="sp", bufs=2, space="PSUM"))
        op = ctx.enter_context(tc.tile_pool(name="op", bufs=2, space="PSUM"))
        dn = ctx.enter_context(tc.tile_pool(name="dn", bufs=2, space="PSUM"))

        # ---- constants ---------------------------------------------------
        def load_const(dram, shape, dt, tag):
            t = const.tile(shape, dt, tag=tag, name=tag)
            nc.sync.dma_start(out=t[:], in_=dram.ap())
            return t

        ones_col = const.tile([128, 1], BF16, tag="ones_c", name="ones_c")
        nc.vector.memset(ones_col[:], 1.0)
        ones_row = const.tile([1, 128], BF16, tag="ones_r", name="ones_r")
        nc.vector.memset(ones_row[:], 1.0)

        # persistent activation tiles --------------------------------------
        qt = [[perm.tile([128, 512], F32R, tag=f"qt{m}_{j}", name=f"qt{m}_{j}")
               for j in range(NJ)] for m in range(2)]
        kt = [[perm.tile([128, 512], F32R, tag=f"kt{m}_{j}", name=f"kt{m}_{j}")
               for j in range(NJ)] for m in range(2)]
        ot = [[perm.tile([128, 512], BF16, tag=f"ot{c}_{j}", name=f"ot{c}_{j}")
               for j in range(NJ)] for c in range(2)]
        vt = [perm.tile([128, GH, HD], BF16, tag=f"vt{t}", name=f"vt{t}")
              for t in range(ST)]

        # ---- QK projections (fp8 DoubleRow) ------------------------------
        wk_sb = load_const(wk8, [128, NPAIR, 2, 2, 128], FP8, "wk")
        bk_sb = const.tile([128, 2], F32, tag="bk", name="bk")
        nc.sync.dma_start(out=bk_sb[:], in_=bk.ap().rearrange("(m p) -> p m", p=128))

        def load_x8(dram, tag):
            tiles = []
            for p in range(NPAIR):
                t = xch.tile([128, 2, S], FP8, tag="x8", name=tag)
                nc.sync.dma_start(out=t[:], in_=dram[:, p, :, :])
                tiles.append(t)
            return tiles

        def proj_qk(xt, w_sb, b_sb, dst):
            for m in range(2):
                for j in range(NJ):
                    ps = op.tile([128, 512], F32, tag="op", name="pqk")
                    for p in range(NPAIR):
                        nc.tensor.matmul(
                            ps[:],
                            w_sb[:, p, :, m, :],
                            xt[p][:, :, 512 * j:512 * (j + 1)],
                            start=(p == 0), stop=(p == NPAIR - 1),
                            perf_mode=DR)
                    nc.vector.tensor_scalar_add(
                        out=dst[m][j][:], in0=ps[:], scalar1=b_sb[:, m:m + 1])

        xk_t = pre_x[1] if pre_x else load_x8(xk8, "xk")
        wq_sb = load_const(wq8, [128, NPAIR, 2, 2, 128], FP8, "wq")
        bq_sb = const.tile([128, 2], F32, tag="bq", name="bq")
        nc.sync.dma_start(out=bq_sb[:], in_=bq.ap().rearrange("(m p) -> p m", p=128))
        proj_qk(xk_t, wk_sb, bk_sb, kt)
        xq_t = pre_x[0] if pre_x else load_x8(xq8, "xq")
        proj_qk(xq_t, wq_sb, bq_sb, qt)

        # ---- V projection (bf16) -----------------------------------------
        wv_sb = load_const(wv16, [128, DCH, GC], BF16, "wv")
        bv_sb = load_const(bv16, [1, GC], BF16, "bv")
        if pre_x:
            xv_t = pre_x[2]
        else:
            xv_t = []
            for d in range(DCH):
                t = xch.tile([128, S], BF16, tag="xv", name="xv")
                nc.sync.dma_start(out=t[:], in_=xv16[:, d, :])
                xv_t.append(t)
        for t in range(ST):
            ps = dn.tile([128, GC], F32, tag="dn", name="pv")
            for d in range(DCH):
                nc.tensor.matmul(
                    ps[:],
                    xv_t[d][:, 128 * t:128 * (t + 1)],
                    wv_sb[:, d, :],
                    start=(d == 0), stop=False)
            nc.tensor.matmul(ps[:], ones_row[:], bv_sb[:], start=False, stop=True)
            nc.vector.tensor_copy(
                out=vt[t][:], in_=ps[:].rearrange("p (h e) -> p h e", h=GH))

        wo_sb = load_const(wo16, [128, 2, D], BF16, "wo")

        # ---- attention ---------------------------------------------------
        for j in range(NJ):
            nch = 4 * (j + 1)
            dnt = dn.tile([128, 512], F32, tag="dn", name="dnt")
            otp = [op.tile([128, 512], F32, tag="op", name=f"otp{c}")
                   for c in range(2)]
            def emit_qk_act(i):
                """Scores + exp + causal mask for chunk i; returns (pts, off, i)."""
                o = i - 4 * j
                off = 0 if o < 0 else min(128 * o, 256)
                pts = []
                for c in range(2):
                    spt = sp.tile([128, 1024], F32, tag="sp", name="spt")
                    for h2 in range(2):
                        nc.tensor.matmul(
                            spt[:, 512 * h2 + off:512 * (h2 + 1)],
                            kt[c][i // 4][64 * h2:64 * (h2 + 1),
                                          128 * (i % 4):128 * (i % 4) + 128],
                            qt[c][j][64 * h2:64 * (h2 + 1), off:512],
                            start=True, stop=True)
                    pt = ptp.tile([128, 1024], BF16, tag="pt", name="ptt")
                    if o == 3:
                        # exact spans: cols [384,512) and [896,1024)
                        for h2 in range(2):
                            nc.scalar.activation(
                                out=pt[:, 512 * h2 + 384:512 * (h2 + 1)],
                                in_=spt[:, 512 * h2 + 384:512 * (h2 + 1)],
                                func=AF.Exp, scale=SCALE)
                    else:
                        nc.scalar.activation(
                            out=pt[:, off:1024], in_=spt[:, off:1024],
                            func=AF.Exp, scale=SCALE)
                    if o >= 0:
                        # zero the sub-diagonal triangle in both head halves
                        w = 512 - off if o == 3 else 128
                        base = 0 if o < 3 else off - 128 * o
                        nc.gpsimd.affine_select(
                            out=pt[:].rearrange("p (h q) -> p h q", h=2)
                                   [:, :, off:off + w],
                            in_=pt[:].rearrange("p (h q) -> p h q", h=2)
                                   [:, :, off:off + w],
                            compare_op=ALU.is_ge, fill=0.0, base=base,
                            pattern=[[0, 2], [1, w]], channel_multiplier=-1)
                    pts.append(pt)
                    if dbg is not None and j == 0 and c == 0:
                        nc.sync.dma_start(out=dbg["pt"][:, i, :], in_=pt[:])
                return pts, off, i

            def emit_pv(state):
                """P@V + denominators for a previously scored chunk."""
                pts, off, i = state
                for c in range(2):
                    for h2 in range(2):
                        nc.tensor.matmul(
                            otp[c][64 * h2:64 * (h2 + 1), off:512],
                            vt[i][:, 2 * c + h2, :],
                            pts[c][:, 512 * h2 + off:512 * (h2 + 1)],
                            start=(i == 0), stop=(i == nch - 1),
                            skip_group_check=True)
                for c in range(2):
                    for h2 in range(2):
                        nc.tensor.matmul(
                            dnt[64 * c + 32 * h2:64 * c + 32 * h2 + 1, off:512],
                            ones_col[:],
                            pts[c][:, 512 * h2 + off:512 * (h2 + 1)],
                            start=(i == 0), stop=(i == nch - 1),
                            skip_group_check=True,
                            tile_position=(0, 64 * c + 32 * h2))

            # software pipeline: P@V runs one chunk behind QK^T/exp so the
            # in-order PE never waits on ACT/GpSimd for the current chunk
            prev = None
            for i in range(nch):
                cur = emit_qk_act(i)
                if prev is not None:
                    emit_pv(prev)
                prev = cur
            emit_pv(prev)
            # normalize: O^T / denom -> ot (bf16)
            if dbg is not None and j == 0:
                dcp = dbgp.tile([128, 512], F32, tag="dcp", name="dcp")
                nc.vector.tensor_copy(out=dcp[:], in_=dnt[:])
                nc.sync.dma_start(out=dbg["dn0"].ap(), in_=dcp[:])
                for c in range(2):
                    ocp = dbgp.tile([128, 512], F32, tag=f"ocp{c}", name="ocp")
                    nc.vector.tensor_copy(out=ocp[:], in_=otp[c][:])
                    nc.sync.dma_start(out=dbg["op0"][:, c, :], in_=ocp[:])
            rcs = []
            for hh in range(4):
                rc = nrm.tile([1, 512], F32, tag=f"rc{hh}", name=f"rc{hh}")
                nc.vector.reciprocal(rc[:], dnt[32 * hh:32 * hh + 1, :])
                rcs.append(rc)
            if dbg is not None and j == 0:
                for hh in range(4):
                    nc.sync.dma_start(out=dbg["rc0"][hh:hh + 1, :], in_=rcs[hh][:])
            for c in range(2):
                for h2 in range(2):
                    bc = nrm.tile([64, 512], F32, tag=f"bc{h2}", name="bc")
                    nc.gpsimd.partition_broadcast(bc[:], rcs[2 * c + h2][:])
                    nc.vector.tensor_tensor(
                        out=ot[c][j][64 * h2:64 * (h2 + 1), :],
                        in0=otp[c][64 * h2:64 * (h2 + 1), :],
                        in1=bc[:], op=ALU.mult)

        if dbg is not None:
            for m in range(2):
                for j in range(NJ):
                    nc.sync.dma_start(
                        out=dbg["qt"][:, m, 512 * j:512 * (j + 1)],
                        in_=qt[m][j][:].bitcast(F32))
                    nc.sync.dma_start(
                        out=dbg["kt"][:, m, 512 * j:512 * (j + 1)],
                        in_=kt[m][j][:].bitcast(F32))
                    nc.sync.dma_start(
                        out=dbg["ot"][:, m, 512 * j:512 * (j + 1)],
                        in_=ot[m][j][:])
            for t in range(ST):
                nc.sync.dma_start(out=dbg["vt"][:, t, :, :], in_=vt[t][:])

        # ---- output projection -------------------------------------------
        for j in range(NJ):
            for tt in range(4):
                t = 4 * j + tt
                ob = outp.tile([128, D], F32, tag="ob", name="ob")
                for n2 in range(2):
                    fpt = op.tile([128, 512], F32, tag="op", name="fpt")
                    for c in range(2):
                        nc.tensor.matmul(
                            fpt[:],
                            ot[c][j][:, 128 * tt:128 * (tt + 1)],
                            wo_sb[:, c, 512 * n2:512 * (n2 + 1)],
                            start=(c == 0), stop=(c == 1))
                    nc.vector.tensor_copy(out=ob[:, 512 * n2:512 * (n2 + 1)], in_=fpt[:])
                nc.sync.dma_start(out=out[128 * t:128 * (t + 1), :], in_=ob[:])


_PROGRAMS = {}


def _build_program(reps=1, loop=0, debug_dumps=False, hoist_x=False):
    nc = bacc.Bacc("TRN2", target_bir_lowering=False, debug=False,
                   num_devices=NCORES)
    xq8 = nc.declare_dram_parameter("xq8", [128, NPAIR, 2, S], FP8, isOutput=False)
    xk8 = nc.declare_dram_parameter("xk8", [128, NPAIR, 2, S], FP8, isOutput=False)
    xv16 = nc.declare_dram_parameter("xv16", [128, DCH, S], BF16, isOutput=False)
    wq8 = nc.declare_dram_parameter("wq8", [128, NPAIR, 2, 2, 128], FP8, isOutput=False)
    wk8 = nc.declare_dram_parameter("wk8", [128, NPAIR, 2, 2, 128], FP8, isOutput=False)
    wv16 = nc.declare_dram_parameter("wv16", [128, DCH, GC], BF16, isOutput=False)
    bq = nc.declare_dram_parameter("bq", [GC], F32, isOutput=False)
    bk = nc.declare_dram_parameter("bk", [GC], F32, isOutput=False)
    bv16 = nc.declare_dram_parameter("bv16", [1, GC], BF16, isOutput=False)
    wo16 = nc.declare_dram_parameter("wo16", [128, 2, D], BF16, isOutput=False)
    out = nc.declare_dram_parameter("out_p", [S, D], F32, isOutput=True)
    dbg = None
    if debug_dumps:
        dbg = {
            "qt": nc.declare_dram_parameter("dbg_qt", [128, 2, S], F32, isOutput=True),
            "kt": nc.declare_dram_parameter("dbg_kt", [128, 2, S], F32, isOutput=True),
            "ot": nc.declare_dram_parameter("dbg_ot", [128, 2, S], BF16, isOutput=True),
            "vt": nc.declare_dram_parameter("dbg_vt", [128, ST, GH, HD], BF16, isOutput=True),
            "pt": nc.declare_dram_parameter("dbg_pt", [128, 4, 1024], BF16, isOutput=True),
            "dn0": nc.declare_dram_parameter("dbg_dn0", [128, 512], F32, isOutput=True),
            "rc0": nc.declare_dram_parameter("dbg_rc0", [4, 512], F32, isOutput=True),
            "op0": nc.declare_dram_parameter("dbg_op0", [128, 2, 512], F32, isOutput=True),
        }
    with tile.TileContext(nc) as tc:
        pre_x = None
        if hoist_x:
            with tc.tile_pool(name="prex", bufs=8) as prex:
                xq_t, xk_t = [], []
                for dram, lst in ((xq8, xq_t), (xk8, xk_t)):
                    for p in range(NPAIR):
                        t = prex.tile([128, 2, S], FP8, tag="x8", name="px8")
                        nc.sync.dma_start(out=t[:], in_=dram[:, p, :, :])
                        lst.append(t)
                xv_t = []
                for d in range(DCH):
                    t = prex.tile([128, S], BF16, tag="xv", name="pxv")
                    nc.sync.dma_start(out=t[:], in_=xv16[:, d, :])
                    xv_t.append(t)
                pre_x = (xq_t, xk_t, xv_t)
                if loop:
                    with tc.For_i(0, loop, 1):
                        _emit(tc, xq8, xk8, xv16, wq8, wk8, wv16, bq, bk, bv16,
                              wo16, out, pre_x=pre_x)
                else:
                    for _ in range(reps):
                        _emit(tc, xq8, xk8, xv16, wq8, wk8, wv16, bq, bk, bv16,
                              wo16, out, dbg=dbg, pre_x=pre_x)
        elif loop:
            with tc.For_i(0, loop, 1):
                _emit(tc, xq8, xk8, xv16, wq8, wk8, wv16, bq, bk, bv16, wo16, out)
        else:
            for _ in range(reps):
                _emit(tc, xq8, xk8, xv16, wq8, wk8, wv16, bq, bk, bv16, wo16, out,
                      dbg=dbg)
    nc.compile()
    return nc


def get_program(reps=1, loop=0, debug_dumps=False, hoist_x=False):
    key = (reps, loop, debug_dumps, hoist_x)
    if key not in _PROGRAMS:
        _PROGRAMS[key] = _build_program(reps, loop, debug_dumps, hoist_x)
    return _PROGRAMS[key]


def _to_fp8_pairs(xT):
    # xT: (D, S) f32 -> [128, NPAIR, 2, S] fp8e4: out[k, p, s, t] = xT[256p+128s+k, t]
    a = xT.reshape(NPAIR, 2, 128, -1).transpose(2, 0, 1, 3)
    return np.ascontiguousarray(a.astype(ml_dtypes.float8_e4m3fn))


def _w_fp8_pairs(W):
    # W: (D, GC) f32 -> [128, NPAIR, 2, 2, 128]: out[k,p,s,m,c] = W[256p+128s+k, 128m+c]
    a = W.reshape(NPAIR, 2, 128, 2, 128).transpose(2, 0, 1, 3, 4)
    return np.ascontiguousarray(a.astype(ml_dtypes.float8_e4m3fn))


def make_in_maps(inputs):
    q = np.asarray(inputs["query"], np.float32)
    k = np.asarray(inputs["key"], np.float32)
    v = np.asarray(inputs["value"], np.float32)
    Wq = np.asarray(inputs["Wq"], np.float32)
    Wk = np.asarray(inputs["Wk"], np.float32)
    Wv = np.asarray(inputs["Wv"], np.float32)
    bq = np.asarray(inputs["bq"], np.float32)
    bk = np.asarray(inputs["bk"], np.float32)
    bv = np.asarray(inputs["bv"], np.float32)
    Wo = np.asarray(inputs["Wo"], np.float32)
    bf16 = ml_dtypes.bfloat16
    xq8 = [_to_fp8_pairs(q[b].T) for b in range(B)]
    xk8 = [_to_fp8_pairs(k[b].T) for b in range(B)]
    xv16 = [np.ascontiguousarray(
        v[b].T.reshape(DCH, 128, S).transpose(1, 0, 2).astype(bf16))
        for b in range(B)]
    in_maps = []
    for core in range(NCORES):
        b, g = divmod(core, NGROUPS)
        cs = slice(g * GC, (g + 1) * GC)
        in_maps.append({
            "xq8": xq8[b], "xk8": xk8[b], "xv16": xv16[b],
            "wq8": _w_fp8_pairs(Wq[:, cs]),
            "wk8": _w_fp8_pairs(Wk[:, cs]),
            "wv16": np.ascontiguousarray(
                Wv[:, cs].reshape(DCH, 128, GC).transpose(1, 0, 2).astype(bf16)),
            "bq": np.ascontiguousarray(bq[cs]),
            "bk": np.ascontiguousarray(bk[cs]),
            "bv16": np.ascontiguousarray(bv[cs].reshape(1, GC).astype(bf16)),
            "wo16": np.ascontiguousarray(
                Wo[cs, :].reshape(2, 128, D).transpose(1, 0, 2).astype(bf16)),
        })
    return in_maps


def combine_outputs(results, inputs):
    bo = np.asarray(inputs["bo"], np.float32)
    out = np.zeros((B, S, D), np.float32)
    for core in range(NCORES):
        out[core // NGROUPS] += results[core]["out_p"]
    out += bo
    return out


def kernel(**inputs):
    nc = get_program()
    in_maps = make_in_maps(inputs)
    res = run_bass_kernel_spmd(nc, in_maps, core_ids=list(range(NCORES)))
    return combine_outputs(res.results, inputs)


# revision 32
# speedup vs baseline: 1.4031x; 1.0329x over previous
"""Distributed causal multi-head attention for one TRN2 chip (8 NeuronCores).

Sharding: 2 batches x 4 head-groups. Core c handles batch c//4 and heads
[4*(c%4), 4*(c%4)+4). Wq/Wk/Wv are column-sliced per head group, Wo is
row-sliced; each core emits a partial (S, D) output and the host sums the
4 partials per batch and adds bo. No on-device collectives.

Per-core pipeline (PE cost = moving-dim rows; K/M idle halves recovered
via 32-aligned row/col tiling so paired matmuls run concurrently):
  Q^T,K^T = Wq/Wk-pairs.T @ X^T-pairs   fp8e4 DoubleRow (2 k-chunks/mm)
  V       = X^T-chunk.T @ Wv            bf16 (+ rank-1 bias matmul)
  S^T     = K_h^T.T @ Q_h^T             f32r, head pair row-tiled (0/64)
  P       = exp(scale * S^T)            ACT, one call per chunk (both heads)
  O^T     = V_h.T @ P_h                 bf16, head pair col-tiled (0/64)
  denom   = 1.T @ P_h                   bf16, 4 heads col-tiled (0/32/64/96)
  out     = (O^T/denom).T @ Wo          bf16 stationary, f32 out
"""

import math
import os
import sys

import numpy as np

for _p in ("/opt/trn_rl_repo", "/root/.axon_site/_ro/trn_rl_repo"):
    if os.path.isdir(_p) and _p not in sys.path:
        sys.path.insert(0, _p)

import ml_dtypes

import concourse.bass as bass
import concourse.mybir as mybir
import concourse.tile as tile
from concourse import bacc
from concourse.bass_utils import run_bass_kernel_spmd

F32 = mybir.dt.float32
F32R = mybir.dt.float32r
BF16 = mybir.dt.bfloat16
FP8 = mybir.dt.float8e4
AF = mybir.ActivationFunctionType
ALU = mybir.AluOpType
DR = mybir.MatmulPerfMode.DoubleRow

B, S, D = 2, 2048, 1024
H, HD = 16, 64
NCORES, NGROUPS = 8, 4
GC = D // NGROUPS            # 256 attention columns per core (4 heads)
GH = GC // HD                # 4 heads per core (2 pairs)
SCALE = 1.0 / math.sqrt(S)   # reference divides by sqrt(Sk), not sqrt(hd)
DCH = D // 128               # 8 contraction chunks over model dim
NPAIR = DCH // 2             # 4 DoubleRow k-chunk pairs
ST = S // 128                # 16 key chunks
NJ = S // 512                # 4 query blocks


def _emit(tc, xq8, xk8, xv16, wq8, wk8, wv16, bq, bk, bv16, wo16, out, dbg=None,
          pre_x=None):
    nc = tc.nc
    from contextlib import ExitStack

    with ExitStack() as ctx:
        const = ctx.enter_context(tc.tile_pool(name="const", bufs=1))
        perm = ctx.enter_context(tc.tile_pool(name="perm", bufs=1))
        xch = ctx.enter_context(tc.tile_pool(name="xch", bufs=8))
        ptp = ctx.enter_context(tc.tile_pool(name="ptp", bufs=6))
        nrm = ctx.enter_context(tc.tile_pool(name="nrm", bufs=4))
        outp = ctx.enter_context(tc.tile_pool(name="outp", bufs=3))
        # PSUM: sp 2-bank tiles x2 + op 1x2 + dn 1x1 + pp 1x1 = 8 banks
        if dbg is not None:
            dbgp = ctx.enter_context(tc.tile_pool(name="dbgp", bufs=1))
        sp = ctx.enter_context(tc.tile_pool(name="sp", bufs=2, space="PSUM"))
        op = ctx.enter_context(tc.tile_pool(name="op", bufs=2, space="PSUM"))
        dn = ctx.enter_context(tc.tile_pool(name="dn", bufs=1, space="PSUM"))
        pp = ctx.enter_context(tc.tile_pool(name="pp", bufs=1, space="PSUM"))

        # ---- constants ---------------------------------------------------
        def load_const(dram, shape, dt, tag):
            t = const.tile(shape, dt, tag=tag, name=tag)
            nc.sync.dma_start(out=t[:], in_=dram.ap())
            return t

        ones_col = const.tile([128, 1], BF16, tag="ones_c", name="ones_c")
        nc.vector.memset(ones_col[:], 1.0)
        ones_row = const.tile([1, 128], BF16, tag="ones_r", name="ones_r")
        nc.vector.memset(ones_row[:], 1.0)

        # persistent activation tiles --------------------------------------
        qt = [[perm.tile([128, 512], F32R, tag=f"qt{m}_{j}", name=f"qt{m}_{j}")
               for j in range(NJ)] for m in range(2)]
        kt = [[perm.tile([128, 512], F32R, tag=f"kt{m}_{j}", name=f"kt{m}_{j}")
               for j in range(NJ)] for m in range(2)]
        ot = [[perm.tile([128, 512], BF16, tag=f"ot{c}_{j}", name=f"ot{c}_{j}")
               for j in range(NJ)] for c in range(2)]
        vt = [perm.tile([128, GH, HD], BF16, tag=f"vt{t}", name=f"vt{t}")
              for t in range(ST)]

        # ---- QK projections (fp8 DoubleRow) ------------------------------
        wk_sb = load_const(wk8, [128, NPAIR, 2, 2, 128], FP8, "wk")
        bk_sb = const.tile([128, 2], F32, tag="bk", name="bk")
        nc.sync.dma_start(out=bk_sb[:], in_=bk.ap().rearrange("(m p) -> p m", p=128))

        def load_x8(dram, tag):
            tiles = []
            for p in range(NPAIR):
                t = xch.tile([128, 2, S], FP8, tag="x8", name=tag)
                nc.sync.dma_start(out=t[:], in_=dram[:, p, :, :])
                tiles.append(t)
            return tiles

        xk_t = pre_x[1] if pre_x else load_x8(xk8, "xk")
        wq_sb = load_const(wq8, [128, NPAIR, 2, 2, 128], FP8, "wq")
        bq_sb = const.tile([128, 2], F32, tag="bq", name="bq")
        nc.sync.dma_start(out=bq_sb[:], in_=bq.ap().rearrange("(m p) -> p m", p=128))
        xq_t = pre_x[0] if pre_x else load_x8(xq8, "xq")

        # ---- V projection inputs (bf16) ----------------------------------
        wv_sb = load_const(wv16, [128, DCH, GC], BF16, "wv")
        bv_sb = load_const(bv16, [1, GC], BF16, "bv")
        if pre_x:
            xv_t = pre_x[2]
        else:
            xv_t = []
            for d in range(DCH):
                t = xch.tile([128, S], BF16, tag="xv", name="xv")
                nc.sync.dma_start(out=t[:], in_=xv16[:, d, :])
                xv_t.append(t)
        wo_sb = load_const(wo16, [128, 2, D], BF16, "wo")

        # ---- projection / output-projection task closures ----------------
        # Each task = one PSUM group + its DVE drain, through the single pp
        # bank; tasks are sprinkled between attention chunks so the PE fills
        # its slack under the ACT (exp) shadow.
        def task_projqk(xt, w_sb, b_sb, dst, m, j):
            def run():
                ps = pp.tile([128, 512], F32, tag="pp", name="pqk")
                for p in range(NPAIR):
                    nc.tensor.matmul(
                        ps[:],
                        w_sb[:, p, :, m, :],
                        xt[p][:, :, 512 * j:512 * (j + 1)],
                        start=(p == 0), stop=(p == NPAIR - 1),
                        perf_mode=DR)
                nc.vector.tensor_scalar_add(
                    out=dst[m][j][:], in0=ps[:], scalar1=b_sb[:, m:m + 1])
            return run

        def task_projv(t):
            def run():
                ps = pp.tile([128, GC], F32, tag="pp", name="pv")
                for d in range(DCH):
                    nc.tensor.matmul(
                        ps[:],
                        xv_t[d][:, 128 * t:128 * (t + 1)],
                        wv_sb[:, d, :],
                        start=(d == 0), stop=False)
                nc.tensor.matmul(ps[:], ones_row[:], bv_sb[:], start=False,
                                 stop=True)
                nc.vector.tensor_copy(
                    out=vt[t][:], in_=ps[:].rearrange("p (h e) -> p h e", h=GH))
            return run

        def task_outproj(j, tt):
            def run():
                t = 4 * j + tt
                ob = outp.tile([128, D], F32, tag="ob", name="ob")
                for n2 in range(2):
                    fpt = pp.tile([128, 512], F32, tag="pp", name="fpt")
                    for c in range(2):
                        nc.tensor.matmul(
                            fpt[:],
                            ot[c][j][:, 128 * tt:128 * (tt + 1)],
                            wo_sb[:, c, 512 * n2:512 * (n2 + 1)],
                            start=(c == 0), stop=(c == 1))
                    nc.vector.tensor_copy(
                        out=ob[:, 512 * n2:512 * (n2 + 1)], in_=fpt[:])
                nc.sync.dma_start(out=out[128 * t:128 * (t + 1), :], in_=ob[:])
            return run

        def proj_tasks(j):
            ts = []
            for m in range(2):
                ts.append(task_projqk(xk_t, wk_sb, bk_sb, kt, m, j))
            for m in range(2):
                ts.append(task_projqk(xq_t, wq_sb, bq_sb, qt, m, j))
            for t in range(4 * j, 4 * j + 4):
                ts.append(task_projv(t))
            return ts

        # ---- attention ---------------------------------------------------
        for t in proj_tasks(0):
            t()
        for j in range(NJ):
            nch = 4 * (j + 1)
            # filler tasks for the PE: project the next block, output-project
            # the previous one
            tasks = list(proj_tasks(j + 1)) if j + 1 < NJ else []
            if j >= 1:
                tasks += [task_outproj(j - 1, tt) for tt in range(4)]
            dnt = dn.tile([128, 512], F32, tag="dn", name="dnt")
            otp = [op.tile([128, 512], F32, tag="op", name=f"otp{c}")
                   for c in range(2)]
            def emit_qk_act(i):
                """Scores + exp + causal mask for chunk i; returns (pts, off, i)."""
                o = i - 4 * j
                off = 0 if o < 0 else min(128 * o, 256)
                pts = []
                for c in range(2):
                    spt = sp.tile([128, 1024], F32, tag="sp", name="spt")
                    for h2 in range(2):
                        nc.tensor.matmul(
                            spt[:, 512 * h2 + off:512 * (h2 + 1)],
                            kt[c][i // 4][64 * h2:64 * (h2 + 1),
                                          128 * (i % 4):128 * (i % 4) + 128],
                            qt[c][j][64 * h2:64 * (h2 + 1), off:512],
                            start=True, stop=True)
                    pt = ptp.tile([128, 1024], BF16, tag="pt", name="ptt")
                    if o == 3:
                        # exact spans: cols [384,512) and [896,1024)
                        for h2 in range(2):
                            nc.scalar.activation(
                                out=pt[:, 512 * h2 + 384:512 * (h2 + 1)],
                                in_=spt[:, 512 * h2 + 384:512 * (h2 + 1)],
                                func=AF.Exp, scale=SCALE)
                    else:
                        nc.scalar.activation(
                            out=pt[:, off:1024], in_=spt[:, off:1024],
                            func=AF.Exp, scale=SCALE)
                    if o >= 0:
                        # zero the sub-diagonal triangle in both head halves
                        w = 512 - off if o == 3 else 128
                        base = 0 if o < 3 else off - 128 * o
                        nc.gpsimd.affine_select(
                            out=pt[:].rearrange("p (h q) -> p h q", h=2)
                                   [:, :, off:off + w],
                            in_=pt[:].rearrange("p (h q) -> p h q", h=2)
                                   [:, :, off:off + w],
                            compare_op=ALU.is_ge, fill=0.0, base=base,
                            pattern=[[0, 2], [1, w]], channel_multiplier=-1)
                    pts.append(pt)
                    if dbg is not None and j == 0 and c == 0:
                        nc.sync.dma_start(out=dbg["pt"][:, i, :], in_=pt[:])
                return pts, off, i

            def emit_pv(state):
                """P@V + denominators for a previously scored chunk."""
                pts, off, i = state
                for c in range(2):
                    for h2 in range(2):
                        nc.tensor.matmul(
                            otp[c][64 * h2:64 * (h2 + 1), off:512],
                            vt[i][:, 2 * c + h2, :],
                            pts[c][:, 512 * h2 + off:512 * (h2 + 1)],
                            start=(i == 0), stop=(i == nch - 1),
                            skip_group_check=True)
                for c in range(2):
                    for h2 in range(2):
                        nc.tensor.matmul(
                            dnt[64 * c + 32 * h2:64 * c + 32 * h2 + 1, off:512],
                            ones_col[:],
                            pts[c][:, 512 * h2 + off:512 * (h2 + 1)],
                            start=(i == 0), stop=(i == nch - 1),
                            skip_group_check=True,
                            tile_position=(0, 64 * c + 32 * h2))

            # software pipeline: P@V runs one chunk behind QK^T/exp so the
            # in-order PE never waits on ACT/GpSimd for the current chunk;
            # filler tasks are spread evenly across the chunk loop
            prev = None
            ti = 0
            for i in range(nch):
                cur = emit_qk_act(i)
                if prev is not None:
                    emit_pv(prev)
                while ti * nch < len(tasks) * (i + 1):
                    tasks[ti]()
                    ti += 1
                prev = cur
            emit_pv(prev)
            while ti < len(tasks):
                tasks[ti]()
                ti += 1
            # normalize: O^T / denom -> ot (bf16)
            if dbg is not None and j == 0:
                dcp = dbgp.tile([128, 512], F32, tag="dcp", name="dcp")
                nc.vector.tensor_copy(out=dcp[:], in_=dnt[:])
                nc.sync.dma_start(out=dbg["dn0"].ap(), in_=dcp[:])
                for c in range(2):
                    ocp = dbgp.tile([128, 512], F32, tag=f"ocp{c}", name="ocp")
                    nc.vector.tensor_copy(out=ocp[:], in_=otp[c][:])
                    nc.sync.dma_start(out=dbg["op0"][:, c, :], in_=ocp[:])
            rcs = []
            for hh in range(4):
                rc = nrm.tile([1, 512], F32, tag=f"rc{hh}", name=f"rc{hh}")
                nc.vector.reciprocal(rc[:], dnt[32 * hh:32 * hh + 1, :])
                rcs.append(rc)
            if dbg is not None and j == 0:
                for hh in range(4):
                    nc.sync.dma_start(out=dbg["rc0"][hh:hh + 1, :], in_=rcs[hh][:])
            for c in range(2):
                for h2 in range(2):
                    bc = nrm.tile([64, 512], F32, tag=f"bc{h2}", name="bc")
                    nc.gpsimd.partition_broadcast(bc[:], rcs[2 * c + h2][:])
                    nc.vector.tensor_tensor(
                        out=ot[c][j][64 * h2:64 * (h2 + 1), :],
                        in0=otp[c][64 * h2:64 * (h2 + 1), :],
                        in1=bc[:], op=ALU.mult)

        if dbg is not None:
            for m in range(2):
                for j in range(NJ):
                    nc.sync.dma_start(
                        out=dbg["qt"][:, m, 512 * j:512 * (j + 1)],
                        in_=qt[m][j][:].bitcast(F32))
                    nc.sync.dma_start(
                        out=dbg["kt"][:, m, 512 * j:512 * (j + 1)],
                        in_=kt[m][j][:].bitcast(F32))
                    nc.sync.dma_start(
                        out=dbg["ot"][:, m, 512 * j:512 * (j + 1)],
                        in_=ot[m][j][:])
            for t in range(ST):
                nc.sync.dma_start(out=dbg["vt"][:, t, :, :], in_=vt[t][:])

        # ---- output projection epilogue (last block) ----------------------
        for tt in range(4):
            task_outproj(NJ - 1, tt)()


_PROGRAMS = {}


def _build_program(reps=1, loop=0, debug_dumps=False, hoist_x=False):
    nc = bacc.Bacc("TRN2", target_bir_lowering=False, debug=False,
                   num_devices=NCORES)
    xq8 = nc.declare_dram_parameter("xq8", [128, NPAIR, 2, S], FP8, isOutput=False)
    xk8 = nc.declare_dram_parameter("xk8", [128, NPAIR, 2, S], FP8, isOutput=False)
    xv16 = nc.declare_dram_parameter("xv16", [128, DCH, S], BF16, isOutput=False)
    wq8 = nc.declare_dram_parameter("wq8", [128, NPAIR, 2, 2, 128], FP8, isOutput=False)
    wk8 = nc.declare_dram_parameter("wk8", [128, NPAIR, 2, 2, 128], FP8, isOutput=False)
    wv16 = nc.declare_dram_parameter("wv16", [128, DCH, GC], BF16, isOutput=False)
    bq = nc.declare_dram_parameter("bq", [GC], F32, isOutput=False)
    bk = nc.declare_dram_parameter("bk", [GC], F32, isOutput=False)
    bv16 = nc.declare_dram_parameter("bv16", [1, GC], BF16, isOutput=False)
    wo16 = nc.declare_dram_parameter("wo16", [128, 2, D], BF16, isOutput=False)
    out = nc.declare_dram_parameter("out_p", [S, D], F32, isOutput=True)
    dbg = None
    if debug_dumps:
        dbg = {
            "qt": nc.declare_dram_parameter("dbg_qt", [128, 2, S], F32, isOutput=True),
            "kt": nc.declare_dram_parameter("dbg_kt", [128, 2, S], F32, isOutput=True),
            "ot": nc.declare_dram_parameter("dbg_ot", [128, 2, S], BF16, isOutput=True),
            "vt": nc.declare_dram_parameter("dbg_vt", [128, ST, GH, HD], BF16, isOutput=True),
            "pt": nc.declare_dram_parameter("dbg_pt", [128, 4, 1024], BF16, isOutput=True),
            "dn0": nc.declare_dram_parameter("dbg_dn0", [128, 512], F32, isOutput=True),
            "rc0": nc.declare_dram_parameter("dbg_rc0", [4, 512], F32, isOutput=True),
            "op0": nc.declare_dram_parameter("dbg_op0", [128, 2, 512], F32, isOutput=True),
        }
    with tile.TileContext(nc) as tc:
        pre_x = None
        if hoist_x:
            with tc.tile_pool(name="prex", bufs=8) as prex:
                xq_t, xk_t = [], []
                for dram, lst in ((xq8, xq_t), (xk8, xk_t)):
                    for p in range(NPAIR):
                        t = prex.tile([128, 2, S], FP8, tag="x8", name="px8")
                        nc.sync.dma_start(out=t[:], in_=dram[:, p, :, :])
                        lst.append(t)
                xv_t = []
                for d in range(DCH):
                    t = prex.tile([128, S], BF16, tag="xv", name="pxv")
                    nc.sync.dma_start(out=t[:], in_=xv16[:, d, :])
                    xv_t.append(t)
                pre_x = (xq_t, xk_t, xv_t)
                if loop:
                    with tc.For_i(0, loop, 1):
                        _emit(tc, xq8, xk8, xv16, wq8, wk8, wv16, bq, bk, bv16,
                              wo16, out, pre_x=pre_x)
                else:
                    for _ in range(reps):
                        _emit(tc, xq8, xk8, xv16, wq8, wk8, wv16, bq, bk, bv16,
                              wo16, out, dbg=dbg, pre_x=pre_x)
        elif loop:
            with tc.For_i(0, loop, 1):
                _emit(tc, xq8, xk8, xv16, wq8, wk8, wv16, bq, bk, bv16, wo16, out)
        else:
            for _ in range(reps):
                _emit(tc, xq8, xk8, xv16, wq8, wk8, wv16, bq, bk, bv16, wo16, out,
                      dbg=dbg)
    nc.compile()
    return nc


def get_program(reps=1, loop=0, debug_dumps=False, hoist_x=False):
    key = (reps, loop, debug_dumps, hoist_x)
    if key not in _PROGRAMS:
        _PROGRAMS[key] = _build_program(reps, loop, debug_dumps, hoist_x)
    return _PROGRAMS[key]


def _to_fp8_pairs(xT):
    # xT: (D, S) f32 -> [128, NPAIR, 2, S] fp8e4: out[k, p, s, t] = xT[256p+128s+k, t]
    a = xT.reshape(NPAIR, 2, 128, -1).transpose(2, 0, 1, 3)
    return np.ascontiguousarray(a.astype(ml_dtypes.float8_e4m3fn))


def _w_fp8_pairs(W):
    # W: (D, GC) f32 -> [128, NPAIR, 2, 2, 128]: out[k,p,s,m,c] = W[256p+128s+k, 128m+c]
    a = W.reshape(NPAIR, 2, 128, 2, 128).transpose(2, 0, 1, 3, 4)
    return np.ascontiguousarray(a.astype(ml_dtypes.float8_e4m3fn))


def make_in_maps(inputs):
    q = np.asarray(inputs["query"], np.float32)
    k = np.asarray(inputs["key"], np.float32)
    v = np.asarray(inputs["value"], np.float32)
    Wq = np.asarray(inputs["Wq"], np.float32)
    Wk = np.asarray(inputs["Wk"], np.float32)
    Wv = np.asarray(inputs["Wv"], np.float32)
    bq = np.asarray(inputs["bq"], np.float32)
    bk = np.asarray(inputs["bk"], np.float32)
    bv = np.asarray(inputs["bv"], np.float32)
    Wo = np.asarray(inputs["Wo"], np.float32)
    bf16 = ml_dtypes.bfloat16
    xq8 = [_to_fp8_pairs(q[b].T) for b in range(B)]
    xk8 = [_to_fp8_pairs(k[b].T) for b in range(B)]
    xv16 = [np.ascontiguousarray(
        v[b].T.reshape(DCH, 128, S).transpose(1, 0, 2).astype(bf16))
        for b in range(B)]
    in_maps = []
    for core in range(NCORES):
        b, g = divmod(core, NGROUPS)
        cs = slice(g * GC, (g + 1) * GC)
        in_maps.append({
            "xq8": xq8[b], "xk8": xk8[b], "xv16": xv16[b],
            "wq8": _w_fp8_pairs(Wq[:, cs]),
            "wk8": _w_fp8_pairs(Wk[:, cs]),
            "wv16": np.ascontiguousarray(
                Wv[:, cs].reshape(DCH, 128, GC).transpose(1, 0, 2).astype(bf16)),
            "bq": np.ascontiguousarray(bq[cs]),
            "bk": np.ascontiguousarray(bk[cs]),
            "bv16": np.ascontiguousarray(bv[cs].reshape(1, GC).astype(bf16)),
            "wo16": np.ascontiguousarray(
                Wo[cs, :].reshape(2, 128, D).transpose(1, 0, 2).astype(bf16)),
        })
    return in_maps


def combine_outputs(results, inputs):
    bo = np.asarray(inputs["bo"], np.float32)
    out = np.zeros((B, S, D), np.float32)
    for core in range(NCORES):
        out[core // NGROUPS] += results[core]["out_p"]
    out += bo
    return out


def kernel(**inputs):
    nc = get_program()
    in_maps = make_in_maps(inputs)
    res = run_bass_kernel_spmd(nc, in_maps, core_ids=list(range(NCORES)))
    return combine_outputs(res.results, inputs)
